# revision 2
# baseline (speedup 1.0000x reference)
"""Trainium2 Bass kernel for the 2-layer ChebConv (K=4) GNN with graph pooling.

v2 strategy (8 NeuronCores, SPMD single program):
  - Nodes sharded into 8 contiguous slabs by destination.  Edge structure
    preprocessed on host into (dest-block x src-chunk) cells; tiles of <=128
    edges whose dests lie in a STATIC 128-wide window of the block (no dynamic
    PSUM offsets -> no PE register loads).
  - Sparse L_hat @ h: dma_gather of source rows (int16 chunk-relative idx)
    from the Shared AllGather output directly (no local copy), PE matmul
    msgs[128,64]^T @ onehot[128,128] accumulated in PSUM [64,512].
  - Source chunks = quarter-slabs of all cores; u propagated with FOUR
    quarter AllGathers per round, fired as soon as the producing dest-quarter
    completes, so next-round gathers overlap this round's tail.
  - Messages cast fp32->bf16 on scalar; one-hots bf16 on vector.
  - deg/dis computed on host; dis folded source-side into u, -dis/-2dis
    dest-side at block finalize; diag of L_hat is 0.
  - Pooling: h2 AllGathered feature-major; per-graph segment reduce with
    compile-time boundaries; linear head on device.
"""

import math
import numpy as np

NC = 8
BLK = 512
WIN = 32
TILE = 128
G_FIXED = 256
NQ = 4  # SWDGE queues
QB = [0, 3072, 6144, 9216, 12500]  # quarter bounds within a slab


def _to_bf16(a):
    import ml_dtypes
    return np.asarray(a, np.float32).astype(ml_dtypes.bfloat16)


def _blocks():
    blocks = []
    for p in range(4):
        s = QB[p]
        while s < QB[p + 1]:
            w = min(BLK, QB[p + 1] - s)
            blocks.append((s, w, p))
            s += w
    return blocks


# ----------------------------------------------------------------------------
# Host-side preprocessing
# ----------------------------------------------------------------------------

def _prep_edges(row, col, ew, N, S):
    qlen = np.array([QB[k + 1] - QB[k] for k in range(4)])
    qbs = np.array(QB[:4])
    blocks = _blocks()
    nblk = len(blocks)
    bstarts = np.array([b[0] for b in blocks])

    ic = row // S
    rl = row - ic * S
    rq = np.searchsorted(QB, rl, side="right") - 1
    rq = np.clip(rq, 0, 3)
    cidx = (ic * qlen[rq] + (rl - qbs[rq])).astype(np.int64)

    # per-core, per-cell dest-sorted edge lists
    cell_e = {}
    cell_d = {}
    for i in range(NC):
        sel = np.nonzero((col // S) == i)[0]
        dloc = (col[sel] - i * S).astype(np.int64)
        order = np.argsort(dloc, kind="stable")
        sel = sel[order]
        dloc = dloc[order]
        bi = np.searchsorted(bstarts, dloc, side="right") - 1
        cq = rq[sel]
        key = bi * 4 + cq
        o2 = np.argsort(key, kind="stable")
        ks = key[o2]
        bounds = np.searchsorted(ks, np.arange(nblk * 4 + 1))
        es = sel[o2]
        ds = (dloc - bstarts[bi])[o2]
        for k in range(nblk * 4):
            lo, hi = int(bounds[k]), int(bounds[k + 1])
            cell_e[(i, k)] = es[lo:hi]
            cell_d[(i, k)] = ds[lo:hi]

    # co-designed greedy cut: shared compile-time window offsets per tile
    WOFF = [[[] for _ in range(4)] for _ in range(nblk)]
    fills = {}  # (b,c) -> list of per-tile [(lo_i,hi_i)]*NC
    for b in range(nblk):
        bw = blocks[b][1]
        wclamp = max(0, bw - WIN)
        for c in range(4):
            k = b * 4 + c
            lists = [cell_d[(i, k)] for i in range(NC)]
            ns = [len(l) for l in lists]
            ptr = [0] * NC
            tl = []
            while any(ptr[i] < ns[i] for i in range(NC)):
                wt = min(lists[i][ptr[i]] for i in range(NC)
                         if ptr[i] < ns[i])
                wt = min(int(wt), wclamp)
                cuts = []
                for i in range(NC):
                    lim = int(np.searchsorted(lists[i], wt + WIN,
                                              side="left"))
                    new = min(ptr[i] + TILE, max(lim, ptr[i]))
                    cuts.append((ptr[i], new))
                    ptr[i] = new
                WOFF[b][c].append(wt)
                tl.append(cuts)
            fills[(b, c)] = tl
    T = [[len(WOFF[b][c]) for c in range(4)] for b in range(nblk)]
    TOT = sum(sum(tb) for tb in T)

    out = []
    for i in range(NC):
        gidx = np.zeros((TOT, TILE), np.int16)
        drel = np.zeros((TILE, TOT), np.uint8)
        ewv = np.zeros((TILE, TOT), np.float32)
        t0 = 0
        for b in range(nblk):
            for c in range(4):
                k = b * 4 + c
                es = cell_e[(i, k)]
                ds = cell_d[(i, k)]
                for t, cuts in enumerate(fills[(b, c)]):
                    lo, hi = cuts[i]
                    if hi > lo:
                        n = hi - lo
                        wt = WOFF[b][c][t]
                        gidx[t0 + t, :n] = cidx[es[lo:hi]].astype(np.int16)
                        drel[:n, t0 + t] = (ds[lo:hi] - wt).astype(np.uint8)
                        ewv[:n, t0 + t] = ew[es[lo:hi]]
                t0 += T[b][c]
        flat = gidx.reshape(-1)
        idx16 = np.zeros((16, TOT * TILE // 16), np.int16)
        ssl = np.arange(TOT * TILE)
        idx16[ssl % 16, ssl // 16] = flat
        idx128 = np.tile(idx16, (8, 1))
        out.append(dict(idx128=idx128, drel=drel, ewv=_to_bf16(ewv)))
    return out, T, WOFF, TOT, blocks


# ----------------------------------------------------------------------------
# Bass program
# ----------------------------------------------------------------------------

def _build(cfg):
    from concourse import bass, bacc, tile, mybir
    from concourse.masks import make_identity
    import contextlib

    f32 = mybir.dt.float32
    bf16 = mybir.dt.bfloat16
    i16 = mybir.dt.int16
    u8 = mybir.dt.uint8

    N, S, F, G = cfg["N"], cfg["S"], cfg["F"], cfg["G"]
    K = cfg["K"]
    T, WOFF, TOT = cfg["T"], cfg["WOFF"], cfg["TOT"]
    blocks = cfg["blocks"]
    nblk = len(blocks)
    NTL = math.ceil(S / TILE)
    gb = cfg["graph_bounds"]
    qlen = [QB[k + 1] - QB[k] for k in range(4)]
    gcount = [0]

    nc = bacc.Bacc("TRN2", target_bir_lowering=False, debug=False,
                   num_devices=NC, num_swdge_queues=NQ)

    def din(name, shape, dt):
        return nc.dram_tensor(name, shape, dt, kind="ExternalInput")

    t_xfm = din("x_fm", [F, S], f32)
    t_idx = din("gidx", [128, TOT * TILE // 16], i16)
    t_drel = din("drel", [TILE, TOT], u8)
    t_ewv = din("ewv", [TILE, TOT], bf16)
    t_dis = din("disp", [1, NTL * TILE], f32)
    t_dism1 = din("dism1", [1, S], f32)
    t_dism2 = din("dism2", [1, S], f32)
    t_w1 = din("w1", [F, K * F], f32)
    t_w2 = din("w2", [F, K * F], f32)
    t_b1 = din("b1c", [F, 1], f32)
    t_b2 = din("b2c", [F, 1], f32)
    t_gam = din("gam", [F, 1], f32)
    t_bet = din("bet", [F, 1], f32)
    t_mu = din("muv", [F, 1], f32)
    t_var = din("varv", [F, 1], f32)
    t_linw = din("linwt", [F, 6], f32)
    t_linb = din("linbc", [2, 1], f32)
    t_cnt = din("cntf", [1, G], f32)
    t_out = nc.dram_tensor("out", [2, G], f32, kind="ExternalOutput")

    Tcmax = max(max(tb) for tb in T)
    t_iota = din("iotap", [TILE, Tcmax * WIN], u8)

    rg = [list(range(NC))]

    with tile.TileContext(nc) as tc:
        ctx = contextlib.ExitStack()
        with ctx:
            sb = ctx.enter_context(tc.tile_pool(name="sb", bufs=1))
            ps = ctx.enter_context(tc.tile_pool(name="ps", bufs=1, space="PSUM"))
            dr = ctx.enter_context(tc.tile_pool(name="dr", bufs=1, space="DRAM"))

            # ---------------- persistent loads ----------------
            iota_sb = sb.tile([TILE, Tcmax * WIN], u8)
            nc.sync.dma_start(out=iota_sb[:], in_=t_iota[:, :])
            ones1f = sb.tile([1, F], f32)
            nc.vector.memset(ones1f[:], 1.0)
            ident = sb.tile([TILE, TILE], f32)
            make_identity(nc, ident[:])
            w1_sb = sb.tile([F, K * F], f32)
            nc.sync.dma_start(out=w1_sb[:], in_=t_w1[:, :])
            w2_sb = sb.tile([F, K * F], f32)
            nc.sync.dma_start(out=w2_sb[:], in_=t_w2[:, :])
            b1_sb = sb.tile([F, 1], f32)
            nc.sync.dma_start(out=b1_sb[:], in_=t_b1[:, :])
            b2_sb = sb.tile([F, 1], f32)
            nc.sync.dma_start(out=b2_sb[:], in_=t_b2[:, :])
            linw_sb = sb.tile([F, 6], f32)
            nc.sync.dma_start(out=linw_sb[:], in_=t_linw[:, :])
            linb_sb = sb.tile([2, 1], f32)
            nc.sync.dma_start(out=linb_sb[:], in_=t_linb[:, :])
            cnt_sb = sb.tile([1, G], f32)
            nc.sync.dma_start(out=cnt_sb[:], in_=t_cnt[:, :])

            gam_sb = sb.tile([F, 1], f32)
            nc.sync.dma_start(out=gam_sb[:], in_=t_gam[:, :])
            bet_sb = sb.tile([F, 1], f32)
            nc.sync.dma_start(out=bet_sb[:], in_=t_bet[:, :])
            mu_sb = sb.tile([F, 1], f32)
            nc.sync.dma_start(out=mu_sb[:], in_=t_mu[:, :])
            var_sb = sb.tile([F, 1], f32)
            nc.sync.dma_start(out=var_sb[:], in_=t_var[:, :])
            bnscale = sb.tile([F, 1], f32)
            bnbias = sb.tile([F, 1], f32)
            tmp1 = sb.tile([F, 1], f32)
            nc.vector.tensor_scalar_add(tmp1[:], var_sb[:], 1e-5)
            nc.vector.reciprocal(tmp1[:], tmp1[:])
            nc.scalar.sqrt(tmp1[:], tmp1[:])
            nc.vector.tensor_mul(bnscale[:], gam_sb[:], tmp1[:])
            nc.vector.tensor_mul(tmp1[:], bnscale[:], mu_sb[:])
            nc.vector.tensor_sub(bnbias[:], bet_sb[:], tmp1[:])

            h1_dram = dr.tile([F, S], f32)
            tx1_dram = dr.tile([F, S], f32)
            # u[r][c]: AllGather output for round r, source chunk c
            u_q = [[dr.tile([NC * qlen[c], F], f32, addr_space="Shared",
                            name=f"u{r}q{c}") for c in range(4)]
                   for r in range(6)]
            h2_full = dr.tile([NC * F, S], bf16, addr_space="Shared",
                              name="h2_full")
            oacc_dram = dr.tile([F, S], f32)

            # dis node-major [128, NTL]: element (p, c) = dis[c*128 + p]
            dis_nm = sb.tile([TILE, NTL], f32)
            nc.sync.dma_start(
                out=dis_nm[:],
                in_=t_dis[0:1, :].rearrange("o (c p) -> (o p) c", p=TILE))

            # ---------------- helpers ----------------
            def build_onehot(oh, drel_t, ew_t, Tb):
                nc.vector.tensor_tensor(
                    out=oh[:, : Tb * WIN],
                    in0=iota_sb[:, : Tb * WIN],
                    in1=drel_t.unsqueeze(-1).to_broadcast([TILE, Tb, WIN]),
                    op=mybir.AluOpType.is_equal)
                nc.vector.tensor_tensor(
                    out=oh[:, : Tb * WIN],
                    in0=oh[:, : Tb * WIN],
                    in1=ew_t.unsqueeze(-1).to_broadcast([TILE, Tb, WIN]),
                    op=mybir.AluOpType.mult)

            # global tile offsets per (b, c)
            cell_off = {}
            t0 = 0
            for b in range(nblk):
                for c in range(4):
                    cell_off[(b, c)] = t0
                    t0 += T[b][c]

            # pending gpsimd collectives, fired N gather-cells after request
            pending_ag = []

            def tick_cell():
                for e in pending_ag:
                    e[1] -= 1
                while pending_ag and pending_ag[0][1] <= 0:
                    pending_ag.pop(0)[0]()

            def flush_ag():
                while pending_ag:
                    pending_ag.pop(0)[0]()

            def ag_quarter(src_tile, rnd_next, p, delay=6):
                """transpose+scale rows [QB[p], QB[p+1]) of src (SBUF [F,S])
                or stream from t_xfm if src_tile is None; AllGather chunk p.
                The collective itself is deferred `delay` gather-cells so it
                never head-of-line-blocks the gpsimd queue."""
                ql = qlen[p]
                ag_in = dr.tile([ql, F], f32, name=f"agin{rnd_next}p{p}")
                ct0 = QB[p] // TILE
                ntl = math.ceil(ql / TILE)
                for j in range(ntl):
                    cs = QB[p] + j * TILE
                    w = min(TILE, QB[p + 1] - cs)
                    if src_tile is None:
                        st = sb.tile([F, TILE], f32, tag="ust", bufs=3)
                        nc.sync.dma_start(out=st[:, :w],
                                          in_=t_xfm[:, cs : cs + w])
                        srcap = st[:, :w]
                    else:
                        srcap = src_tile[:, cs : cs + w]
                    tps = ps.tile([TILE, F], f32, tag="tps", bufs=1)
                    nc.tensor.transpose(out=tps[:w, :F], in_=srcap,
                                        identity=ident[:F, :F])
                    stg = sb.tile([TILE, F], f32, tag="stg", bufs=3)
                    nc.vector.tensor_scalar_mul(stg[:w, :], tps[:w, :F],
                                                dis_nm[:w, ct0 + j : ct0 + j + 1])
                    nc.sync.dma_start(out=ag_in[j * TILE : j * TILE + w, :],
                                      in_=stg[:w, :])

                def fire():
                    nc.gpsimd.collective_compute(
                        "AllGather", mybir.AluOpType.bypass,
                        replica_groups=rg,
                        ins=[ag_in[:]], outs=[u_q[rnd_next][p][:, :]])

                if delay <= 0:
                    fire()
                else:
                    pending_ag.append([fire, delay])

            def conv_block(b, rhs, w_sb, k, first):
                bs, bw, _ = blocks[b]
                cps = ps.tile([F, BLK], f32, tag="cps", bufs=1)
                nc.tensor.matmul(
                    out=cps[:F, :bw],
                    lhsT=w_sb[:, k * F : (k + 1) * F],
                    rhs=rhs,
                    start=True, stop=True)
                st = sb.tile([F, BLK], f32, tag="cst", bufs=2)
                if first:
                    nc.vector.tensor_copy(st[:, :bw], cps[:F, :bw])
                else:
                    nc.sync.dma_start(out=st[:, :bw],
                                      in_=oacc_dram[:, bs : bs + bw])
                    nc.vector.tensor_add(st[:, :bw], st[:, :bw], cps[:F, :bw])
                nc.sync.dma_start(out=oacc_dram[:, bs : bs + bw],
                                  in_=st[:, :bw])

            def seg_round(rnd, dism_tag, sub_src, conv_w, conv_k, txname,
                          produce=None, post=None):
                """One sparse L_hat application.
                produce(p, txt): called after dest-quarter p's blocks final.
                post(b, txt): extra per-block op before produce sees it."""
                txt = sb.tile([F, S], f32, tag="tx", bufs=1, name=txname)
                BAND = 4
                bands = [list(range(s, min(s + BAND, nblk)))
                         for s in range(0, nblk, BAND)]
                last_c = [max((c for c in range(4) if T[b][c] > 0),
                              default=-1) for b in range(nblk)]
                for band in bands:
                    # chunk-major cell sweep: chunk-c3 gathers come late so
                    # the (delayed) AllGather for chunk 3 is already in the
                    # gpsimd stream ahead of them
                    spss = {}
                    for b in band:
                        spss[b] = ps.tile([F, BLK], f32, tag="sps", bufs=4,
                                          name=f"sps_{txname}_{b}")
                        nc.vector.memset(spss[b][:], 0.0)
                    for c in range(4):
                        for b in band:
                            Tc = T[b][c]
                            if Tc == 0:
                                continue
                            tick_cell()
                            bs, bw, bq = blocks[b]
                            sps = spss[b]
                            tcell = cell_off[(b, c)]
                            nidx = Tc * TILE
                            idx_t = sb.tile([128, nidx // 16], i16,
                                            tag="idxc", bufs=6)
                            nc.sync.dma_start(
                                out=idx_t[:],
                                in_=t_idx[:, tcell * 8 :
                                          tcell * 8 + nidx // 16])
                            ms = sb.tile([TILE, Tc, F], f32, tag="ms",
                                         bufs=6)
                            qn = gcount[0] % NQ
                            gcount[0] += 1
                            nc.gpsimd.dma_gather(
                                ms[:],
                                u_q[rnd][c][:, :],
                                idx_t[:, :],
                                nidx, nidx, F,
                                single_packet=False,
                                queue_num=qn)
                            ms_bf = sb.tile([TILE, Tc, F], bf16, tag="msb",
                                            bufs=6)
                            nc.scalar.activation(
                                out=ms_bf[:], in_=ms[:],
                                func=mybir.ActivationFunctionType.Copy,
                                bias=0.0, scale=1.0)
                            drel_t = sb.tile([TILE, Tc], u8, tag="drel",
                                             bufs=4)
                            ewv_t = sb.tile([TILE, Tc], bf16, tag="ewv",
                                            bufs=4)
                            nc.sync.dma_start(
                                out=drel_t[:],
                                in_=t_drel[:, tcell : tcell + Tc])
                            nc.sync.dma_start(
                                out=ewv_t[:],
                                in_=t_ewv[:, tcell : tcell + Tc])
                            oh = sb.tile([TILE, Tc * WIN], bf16, tag="oh",
                                         bufs=4)
                            build_onehot(oh, drel_t[:], ewv_t[:], Tc)
                            for t in range(Tc):
                                wt = WOFF[b][c][t]
                                wl = min(WIN, bw - wt)
                                last = (c == last_c[b]) and (t == Tc - 1)
                                nc.tensor.matmul(
                                    out=sps[:F, wt : wt + wl],
                                    lhsT=ms_bf[:, t, :],
                                    rhs=oh[:, t * WIN : t * WIN + wl],
                                    start=False, stop=last,
                                    skip_group_check=True)
                    # finalize band blocks: txt = sps * dism - sub
                    for b in band:
                        bs, bw, bq = blocks[b]
                        sps = spss[b]
                        dm_t = sb.tile([1, BLK], f32, tag="dm_t", bufs=3)
                        src = t_dism1 if dism_tag == 1 else t_dism2
                        nc.sync.dma_start(out=dm_t[0:1, :bw],
                                          in_=src[0:1, bs : bs + bw])
                        rep = ps.tile([F, BLK], f32, tag="rep", bufs=1)
                        nc.tensor.matmul(out=rep[:F, :bw], lhsT=ones1f[:],
                                         rhs=dm_t[0:1, :bw], start=True,
                                         stop=True)
                        rep_sb = sb.tile([F, BLK], f32, tag="rep_sb", bufs=2)
                        nc.vector.tensor_copy(rep_sb[:, :bw], rep[:F, :bw])
                        nc.vector.tensor_tensor(
                            out=txt[:, bs : bs + bw],
                            in0=sps[:F, :bw],
                            in1=rep_sb[:, :bw],
                            op=mybir.AluOpType.mult)
                        if sub_src is not None:
                            st = sb.tile([F, BLK], f32, tag="cst", bufs=2)
                            nc.sync.dma_start(out=st[:, :bw],
                                              in_=sub_src[:, bs : bs + bw])
                            nc.vector.tensor_sub(txt[:, bs : bs + bw],
                                                 txt[:, bs : bs + bw],
                                                 st[:, :bw])
                        if post is not None:
                            post(b, txt)
                        if conv_w is not None:
                            conv_block(b, txt[:, bs : bs + bw], conv_w,
                                       conv_k, False)
                        if produce is not None and (b == nblk - 1 or
                                                    blocks[b + 1][2] != bq):
                            produce(bq, txt)
                return txt

            # ---------------- layer 1 ----------------
            for p in range(4):
                ag_quarter(None, 0, p, delay=0)  # u[0] = dis*x
            for b in range(nblk):
                bs, bw, _ = blocks[b]
                rhs_t = sb.tile([F, BLK], f32, tag="crhs", bufs=2)
                nc.sync.dma_start(out=rhs_t[:, :bw],
                                  in_=t_xfm[:, bs : bs + bw])
                conv_block(b, rhs_t[:, :bw], w1_sb, 0, True)

            tx1 = seg_round(0, 1, None, w1_sb, 1, "tx1",
                            produce=lambda p, s: ag_quarter(s, 1, p))
            nc.sync.dma_start(out=tx1_dram[:, :], in_=tx1[:])

            tx2 = seg_round(1, 2, t_xfm, w1_sb, 2, "tx2",
                            produce=lambda p, s: ag_quarter(s, 2, p))

            def l1_post(b, txt):
                bs, bw, _ = blocks[b]
                st = sb.tile([F, BLK], f32, tag="cst", bufs=2)
                nc.sync.dma_start(out=st[:, :bw],
                                  in_=oacc_dram[:, bs : bs + bw])
                # oacc += tx3 @ W1[3], then h1 = bn(relu(oacc + b1))
                cps = ps.tile([F, BLK], f32, tag="cps", bufs=1)
                nc.tensor.matmul(
                    out=cps[:F, :bw],
                    lhsT=w1_sb[:, 3 * F : 4 * F],
                    rhs=txt[:, bs : bs + bw],
                    start=True, stop=True)
                nc.vector.tensor_add(st[:, :bw], st[:, :bw], cps[:F, :bw])
                nc.scalar.activation(
                    out=st[:, :bw], in_=st[:, :bw],
                    func=mybir.ActivationFunctionType.Relu,
                    bias=b1_sb[:, 0:1], scale=1.0)
                nc.scalar.activation(
                    out=txt[:, bs : bs + bw], in_=st[:, :bw],
                    func=mybir.ActivationFunctionType.Identity,
                    bias=bnbias[:, 0:1], scale=bnscale[:, 0:1])
                nc.sync.dma_start(out=h1_dram[:, bs : bs + bw],
                                  in_=txt[:, bs : bs + bw])
                # start layer-2 accumulator: oacc = h1 @ W2[0]
                conv_block(b, txt[:, bs : bs + bw], w2_sb, 0, True)

            # tx3 round: conv_w None (k=3 folded into post together with h1)
            h1t = seg_round(2, 2, tx1_dram, None, 0, "h1t",
                            produce=lambda p, s: ag_quarter(s, 3, p),
                            post=l1_post)

            # ---------------- layer 2 ----------------
            tx1b = seg_round(3, 1, None, w2_sb, 1, "tx1b",
                             produce=lambda p, s: ag_quarter(s, 4, p))
            nc.sync.dma_start(out=tx1_dram[:, :], in_=tx1b[:])

            tx2b = seg_round(4, 2, h1_dram, w2_sb, 2, "tx2b",
                             produce=lambda p, s: ag_quarter(s, 5, p))

            ag2_in = dr.tile([F, S], bf16)

            def l2_post(b, txt):
                bs, bw, _ = blocks[b]
                st = sb.tile([F, BLK], f32, tag="cst", bufs=2)
                nc.sync.dma_start(out=st[:, :bw],
                                  in_=oacc_dram[:, bs : bs + bw])
                cps = ps.tile([F, BLK], f32, tag="cps", bufs=1)
                nc.tensor.matmul(
                    out=cps[:F, :bw],
                    lhsT=w2_sb[:, 3 * F : 4 * F],
                    rhs=txt[:, bs : bs + bw],
                    start=True, stop=True)
                nc.vector.tensor_add(st[:, :bw], st[:, :bw], cps[:F, :bw])
                nc.scalar.activation(
                    out=txt[:, bs : bs + bw], in_=st[:, :bw],
                    func=mybir.ActivationFunctionType.Relu,
                    bias=b2_sb[:, 0:1], scale=1.0)
                hb = sb.tile([F, BLK], bf16, tag="h2b", bufs=2)
                nc.scalar.activation(out=hb[:, :bw],
                                     in_=txt[:, bs : bs + bw],
                                     func=mybir.ActivationFunctionType.Copy,
                                     bias=0.0, scale=1.0)
                nc.sync.dma_start(out=ag2_in[:, bs : bs + bw],
                                  in_=hb[:, :bw])

            h2t = seg_round(5, 2, tx1_dram, None, 0, "h2t", post=l2_post)

            flush_ag()
            nc.gpsimd.collective_compute(
                "AllGather", mybir.AluOpType.bypass, replica_groups=rg,
                ins=[ag2_in[:]], outs=[h2_full[:, :]])

            # ---------------- pooling ----------------
            s_cols = sb.tile([F, G], f32)
            mx_cols = sb.tile([F, G], f32)
            nc.vector.memset(s_cols[:], 0.0)
            nc.vector.memset(mx_cols[:], -1e30)
            t_acc = sb.tile([F, 1], f32)
            t_m = sb.tile([F, 1], f32)
            for c in range(NC):
                hch = sb.tile([F, S], bf16, tag="tx", bufs=1, name=f"hch{c}")
                nc.sync.dma_start(out=hch[:], in_=h2_full[c * F : (c + 1) * F, :])
                lo_n, hi_n = c * S, (c + 1) * S
                g_lo = max(int(np.searchsorted(gb, lo_n, side="right")) - 1, 0)
                for g in range(g_lo, G):
                    if int(gb[g]) >= hi_n:
                        break
                    a = max(int(gb[g]), lo_n)
                    b_ = min(int(gb[g + 1]), hi_n)
                    if a >= b_:
                        continue
                    al, bl = a - lo_n, b_ - lo_n
                    whole = int(gb[g]) >= lo_n and int(gb[g + 1]) <= hi_n
                    if whole:
                        nc.vector.tensor_reduce(
                            out=s_cols[:, g : g + 1], in_=hch[:, al:bl],
                            axis=mybir.AxisListType.X, op=mybir.AluOpType.add)
                        nc.vector.tensor_reduce(
                            out=mx_cols[:, g : g + 1], in_=hch[:, al:bl],
                            axis=mybir.AxisListType.X, op=mybir.AluOpType.max)
                    else:
                        nc.vector.tensor_reduce(
                            out=t_acc[:, 0:1], in_=hch[:, al:bl],
                            axis=mybir.AxisListType.X, op=mybir.AluOpType.add)
                        nc.vector.tensor_add(s_cols[:, g : g + 1],
                                             s_cols[:, g : g + 1], t_acc[:, 0:1])
                        nc.vector.tensor_reduce(
                            out=t_m[:, 0:1], in_=hch[:, al:bl],
                            axis=mybir.AxisListType.X, op=mybir.AluOpType.max)
                        nc.vector.tensor_tensor(
                            out=mx_cols[:, g : g + 1], in0=mx_cols[:, g : g + 1],
                            in1=t_m[:, 0:1], op=mybir.AluOpType.max)

            rc = sb.tile([1, G], f32)
            nc.vector.tensor_scalar_max(rc[:], cnt_sb[:], 1.0)
            nc.vector.reciprocal(rc[:], rc[:])
            mean_cols = sb.tile([F, G], f32)
            rep2 = ps.tile([F, G], f32, tag="rep", bufs=1)
            nc.tensor.matmul(out=rep2[:F, :G], lhsT=ones1f[:],
                             rhs=rc[0:1, :], start=True, stop=True)
            nc.vector.tensor_tensor(out=mean_cols[:], in0=s_cols[:],
                                    in1=rep2[:F, :G], op=mybir.AluOpType.mult)
            mk = sb.tile([1, G], f32)
            nc.vector.tensor_scalar(out=mk[:], in0=cnt_sb[:], scalar1=0.0,
                                    scalar2=None, op0=mybir.AluOpType.is_gt)
            rep3 = ps.tile([F, G], f32, tag="rep", bufs=1)
            nc.tensor.matmul(out=rep3[:F, :G], lhsT=ones1f[:],
                             rhs=mk[0:1, :], start=True, stop=True)
            nc.vector.tensor_tensor(out=mx_cols[:], in0=mx_cols[:],
                                    in1=rep3[:F, :G], op=mybir.AluOpType.mult)

            hps = ps.tile([2, G], f32, tag="hps")
            for ci, pc in enumerate([s_cols, mean_cols, mx_cols]):
                nc.tensor.matmul(out=hps[:2, :G],
                                 lhsT=linw_sb[:, 2 * ci : 2 * ci + 2],
                                 rhs=pc[:],
                                 start=(ci == 0), stop=(ci == 2))
            outsb = sb.tile([2, G], f32)
            nc.scalar.activation(out=outsb[:], in_=hps[:2, :G],
                                 func=mybir.ActivationFunctionType.Identity,
                                 bias=linb_sb[:, 0:1], scale=1.0)
            nc.sync.dma_start(out=t_out[:, :], in_=outsb[:])

    nc.compile()
    return nc


# ----------------------------------------------------------------------------
# Entry point
# ----------------------------------------------------------------------------

def _run(x, edge_index, edge_weight, batch, W1, b1, bn_gamma, bn_beta,
         bn_mean, bn_var, W2, b2, linW, linb, G):
    from concourse.bass_utils import run_bass_kernel_spmd

    x = np.asarray(x)
    edge_index = np.asarray(edge_index)
    ew = np.asarray(edge_weight, dtype=np.float32)
    batch = np.asarray(batch)
    N, F = x.shape
    K = int(np.asarray(W1).shape[0])
    S = N // NC

    row = edge_index[0].astype(np.int64)
    col = edge_index[1].astype(np.int64)

    eprep, T, WOFF, TOT, blocks = _prep_edges(row, col, ew, N, S)
    deg = np.zeros(N, np.float32)
    np.add.at(deg, row, ew)
    dis = np.where(deg > 0,
                   1.0 / np.sqrt(np.maximum(deg, 1e-30)), 0.0).astype(np.float32)
    NTL = math.ceil(S / TILE)
    gb = np.searchsorted(batch, np.arange(G + 1))
    cnt = (gb[1:] - gb[:-1]).astype(np.float32)

    Tcmax = max(max(tb) for tb in T)
    iota = np.tile((np.arange(Tcmax * WIN) % WIN).astype(np.uint8),
                   (TILE, 1))

    cfg = dict(N=N, S=S, F=F, G=G, K=K, T=T, WOFF=WOFF, TOT=TOT, blocks=blocks,
               graph_bounds=gb)
    nc = _build(cfg)

    W1a = np.asarray(W1, np.float32)
    W2a = np.asarray(W2, np.float32)
    w1in = np.ascontiguousarray(W1a.transpose(1, 0, 2).reshape(F, K * F))
    w2in = np.ascontiguousarray(W2a.transpose(1, 0, 2).reshape(F, K * F))
    linWa = np.asarray(linW, np.float32)
    linwt = np.concatenate([linWa[:, F * c : F * (c + 1)].T
                            for c in range(3)], axis=1)

    in_maps = []
    for i in range(NC):
        ep = eprep[i]
        dis_pad = np.zeros(NTL * TILE, np.float32)
        dis_pad[:S] = dis[i * S : (i + 1) * S]
        in_maps.append({
            "x_fm": np.ascontiguousarray(x[i * S : (i + 1) * S].T.astype(np.float32)),
            "gidx": ep["idx128"],
            "drel": ep["drel"],
            "ewv": ep["ewv"],
            "disp": dis_pad.reshape(1, -1),
            "dism1": (-dis[i * S : (i + 1) * S]).reshape(1, -1),
            "dism2": (-2.0 * dis[i * S : (i + 1) * S]).reshape(1, -1),
            "w1": w1in, "w2": w2in,
            "b1c": np.asarray(b1, np.float32).reshape(F, 1),
            "b2c": np.asarray(b2, np.float32).reshape(F, 1),
            "gam": np.asarray(bn_gamma, np.float32).reshape(F, 1),
            "bet": np.asarray(bn_beta, np.float32).reshape(F, 1),
            "muv": np.asarray(bn_mean, np.float32).reshape(F, 1),
            "varv": np.asarray(bn_var, np.float32).reshape(F, 1),
            "linwt": np.ascontiguousarray(linwt),
            "linbc": np.asarray(linb, np.float32).reshape(2, 1),
            "cntf": cnt.reshape(1, G),
            "iotap": iota,
        })

    res = run_bass_kernel_spmd(nc, in_maps, core_ids=list(range(NC)))
    out = res.results[0]["out"]
    return np.ascontiguousarray(out.T)


def kernel(x, edge_index, edge_weight, batch, W1, b1, bn_gamma, bn_beta,
           bn_mean, bn_var, W2, b2, linW, linb):
    return _run(x, edge_index, edge_weight, batch, W1, b1, bn_gamma, bn_beta,
                bn_mean, bn_var, W2, b2, linW, linb, G_FIXED)
